# revision 3
# baseline (speedup 1.0000x reference)
"""Trainium2 Bass kernel for nn_AblationAttention (sliding-window causal
attention, W=256, with per-head RMSNorm on q/k).

Key math fact: the reference's "genetic fitness" block adds log(fitness)[b,h,q]
to scores — a constant along the softmax (k) axis — so softmax is invariant to
it and the block is a no-op for the output. We compute plain sliding-window
causal attention.

Sharding: 8 cores = batch (2) x head-group (4 groups of 4 heads).
Each core: full x for its batch (pre-transposed on host), column-sliced
wq/wk/wv, row-sliced wo. Host sums the 4 head-group partials per batch.
"""

import sys

sys.path.insert(0, "/opt/trn_rl_repo")

import numpy as np

import concourse.bass as bass
import concourse.tile as tile
from concourse import bacc, mybir
from concourse import bass_utils
from concourse.masks import make_identity

# Problem constants (hardcoded per harness contract)
B, T, E, H, W = 2, 2048, 1024, 16, 256
D = E // H  # 64
NCORES = 8
HG = 4  # head-groups
HPG = H // HG  # heads per core = 4
COLS = HPG * D  # 256
EPS = float(np.finfo(np.float32).eps)
NEG = -30000.0  # additive mask value; exp(NEG) == 0.0 in f32
FP = mybir.dt.float32
NT = T // 128  # 16 query tiles
KW = 384  # key window width in columns (3 tiles)

_cache = {}


def _build():
    nc = bacc.Bacc(
        "TRN2",
        target_bir_lowering=False,
        debug=False,
        enable_asserts=False,
        num_devices=NCORES,
    )
    xT = nc.dram_tensor("xT", [E, T], FP, kind="ExternalInput").ap()
    wq = nc.dram_tensor("wq", [E, COLS], FP, kind="ExternalInput").ap()
    wk = nc.dram_tensor("wk", [E, COLS], FP, kind="ExternalInput").ap()
    wv = nc.dram_tensor("wv", [E, COLS], FP, kind="ExternalInput").ap()
    wo = nc.dram_tensor("wo", [COLS, E], FP, kind="ExternalInput").ap()
    masks = nc.dram_tensor("masks", [3, 128, KW], FP, kind="ExternalInput").ap()
    qn2 = nc.dram_tensor("qn2", [128, 1], FP, kind="ExternalInput").ap()
    kn2 = nc.dram_tensor("kn2", [128, 1], FP, kind="ExternalInput").ap()
    out = nc.dram_tensor("out", [T, E], FP, kind="ExternalOutput").ap()

    with tile.TileContext(nc) as tc:
        with (
            tc.tile_pool(name="singles", bufs=1) as singles,
            tc.tile_pool(name="xin", bufs=2) as xin,
            tc.tile_pool(name="work", bufs=3) as work,
            tc.tile_pool(name="stats", bufs=4) as stats,
            tc.tile_pool(name="outst", bufs=3) as outst,
        ):
            # ---- resident tensors ----
            wq_sb = singles.tile([128, 8, COLS], FP, tag="wq")
            wk_sb = singles.tile([128, 8, COLS], FP, tag="wk")
            wv_sb = singles.tile([128, 8, COLS], FP, tag="wv")
            wo_sb = singles.tile([128, 2, E], FP, tag="wo")
            mask_sb = singles.tile([128, 3, KW], FP, tag="masks")
            qn_sb = singles.tile([128, 1], FP, tag="qn2")
            kn_sb = singles.tile([128, 1], FP, tag="kn2")
            id_sb = singles.tile([128, 128], FP, tag="ident")
            eps_sb = singles.tile([128, 1], FP, tag="eps")
            qT_sb = [singles.tile([128, T], FP, tag=f"qT{i}", name=f"qT{i}") for i in range(2)]
            kT_sb = [singles.tile([128, T], FP, tag=f"kT{i}", name=f"kT{i}") for i in range(2)]
            v_sb = singles.tile([128, NT, COLS], FP, tag="vsb")
            hoT_sb = [singles.tile([128, T], FP, tag=f"hoT{i}", name=f"hoT{i}") for i in range(2)]

            nc.sync.dma_start(out=wq_sb, in_=wq.rearrange("(k p) c -> p k c", p=128))
            nc.sync.dma_start(out=wk_sb, in_=wk.rearrange("(k p) c -> p k c", p=128))
            nc.sync.dma_start(out=wv_sb, in_=wv.rearrange("(k p) c -> p k c", p=128))
            nc.sync.dma_start(out=wo_sb, in_=wo.rearrange("(k p) e -> p k e", p=128))
            nc.sync.dma_start(out=mask_sb, in_=masks.rearrange("m p j -> p m j"))
            nc.sync.dma_start(out=qn_sb, in_=qn2)
            nc.sync.dma_start(out=kn_sb, in_=kn2)
            make_identity(nc, id_sb)
            nc.vector.memset(eps_sb, EPS)

            xT_r = xT.rearrange("(k p) t -> p k t", p=128)

            # ============ Phase A: qkv projections + rmsnorm + transpose ====
            psA_ctx = tc.tile_pool(name="psA", bufs=2, space="PSUM")
            psA = psA_ctx.__enter__()
            for tc_i in range(4):  # chunks of 512 t
                x_t = xin.tile([128, 8, 512], FP, tag="xT")
                nc.sync.dma_start(out=x_t, in_=xT_r[:, :, tc_i * 512 : (tc_i + 1) * 512])
                for ml in range(4):
                    m = tc_i * 4 + ml
                    sl = slice(ml * 128, (ml + 1) * 128)
                    q_ps = psA.tile([128, COLS], FP, tag="q_ps")
                    k_ps = psA.tile([128, COLS], FP, tag="k_ps")
                    v_ps = psA.tile([128, COLS], FP, tag="v_ps")
                    for kc in range(8):
                        st, sp = (kc == 0), (kc == 7)
                        nc.tensor.matmul(q_ps, x_t[:, kc, sl], wq_sb[:, kc, :], start=st, stop=sp)
                        nc.tensor.matmul(k_ps, x_t[:, kc, sl], wk_sb[:, kc, :], start=st, stop=sp)
                        nc.tensor.matmul(v_ps, x_t[:, kc, sl], wv_sb[:, kc, :], start=st, stop=sp)
                    # v: straight copy to sbuf
                    nc.scalar.copy(v_sb[:, m, :], v_ps)
                    # rmsnorm q and k
                    for (t_ps, nrm_tag, w_ap, dst) in (
                        (q_ps, "qn", qn_sb, qT_sb),
                        (k_ps, "kn", kn_sb, kT_sb),
                    ):
                        sq = work.tile([128, COLS], FP, tag="sq")
                        nc.scalar.activation(sq, t_ps, mybir.ActivationFunctionType.Square)
                        ssq = stats.tile([128, HPG], FP, tag="ssq")
                        nc.vector.tensor_reduce(
                            ssq,
                            sq.rearrange("p (g d) -> p g d", g=HPG),
                            axis=mybir.AxisListType.X,
                            op=mybir.AluOpType.add,
                        )
                        sd = stats.tile([128, HPG], FP, tag="sd")
                        nc.scalar.activation(
                            sd, ssq, mybir.ActivationFunctionType.Sqrt,
                            bias=eps_sb, scale=1.0 / D,
                        )
                        rstd = stats.tile([128, HPG], FP, tag="rstd")
                        nc.vector.reciprocal(rstd, sd)
                        nrm = work.tile([128, COLS], FP, tag="nrm")
                        for g in range(HPG):
                            gs = slice(g * D, (g + 1) * D)
                            nc.vector.tensor_scalar_mul(
                                nrm[:, gs], t_ps[:, gs], rstd[:, g : g + 1]
                            )
                        # transpose the two 128-col head-pair blocks
                        for hp in range(2):
                            t_ps2 = psA.tile([128, 128], FP, tag="t_ps")
                            nc.tensor.transpose(
                                t_ps2, nrm[:, hp * 128 : (hp + 1) * 128], id_sb
                            )
                            nc.vector.tensor_scalar_mul(
                                dst[hp][:, m * 128 : (m + 1) * 128], t_ps2, w_ap
                            )

            psA_ctx.__exit__(None, None, None)

            # ============ Phase B: banded attention ========================
            psB_ctx = tc.tile_pool(name="psB", bufs=2, space="PSUM")
            psB = psB_ctx.__enter__()
            for h in range(HPG):
                hp, off = h // 2, (h % 2) * 64
                po = slice(off, off + 64)
                for qt in range(NT):
                    win = 0 if qt < 2 else (qt - 2) * 128
                    mi = min(qt, 2)
                    qsl = slice(qt * 128, (qt + 1) * 128)
                    s_ps = psB.tile([128, KW], FP, tag="s_ps")
                    nc.tensor.matmul(
                        s_ps, qT_sb[hp][po, qsl], kT_sb[hp][po, win : win + KW],
                        start=True, stop=True,
                    )
                    sm = work.tile([128, KW], FP, tag="sm")
                    nc.vector.tensor_add(sm, s_ps, mask_sb[:, mi, :])
                    p = work.tile([128, KW], FP, tag="p")
                    sums = stats.tile([128, 1], FP, tag="sums")
                    nc.scalar.activation(
                        p, sm, mybir.ActivationFunctionType.Exp, accum_out=sums
                    )
                    rec = stats.tile([128, 1], FP, tag="rec")
                    nc.vector.reciprocal(rec, sums)
                    pn = work.tile([128, KW], FP, tag="pn")
                    nc.vector.tensor_scalar_mul(pn, p, rec)
                    o_ps = psB.tile([64, 128], FP, tag="o_ps")
                    for j in range(3):
                        pT_ps = psB.tile([128, 128], FP, tag="pT_ps")
                        nc.tensor.transpose(pT_ps, pn[:, j * 128 : (j + 1) * 128], id_sb)
                        pT = work.tile([128, 128], FP, tag="pT")
                        nc.scalar.copy(pT, pT_ps)
                        kt = win // 128 + j
                        nc.tensor.matmul(
                            o_ps, v_sb[:, kt, h * 64 : (h + 1) * 64], pT,
                            start=(j == 0), stop=(j == 2),
                        )
                    nc.vector.tensor_copy(hoT_sb[hp][po, qsl], o_ps)

            psB_ctx.__exit__(None, None, None)

            # ============ Phase C: output projection =======================
            psC_ctx = tc.tile_pool(name="psC", bufs=2, space="PSUM")
            psC = psC_ctx.__enter__()
            for m in range(NT):
                msl = slice(m * 128, (m + 1) * 128)
                o_sb = outst.tile([128, E], FP, tag="o_sb")
                for nch in range(2):
                    nsl = slice(nch * 512, (nch + 1) * 512)
                    c_ps = psC.tile([128, 512], FP, tag="c_ps")
                    for c in range(2):
                        nc.tensor.matmul(
                            c_ps, hoT_sb[c][:, msl], wo_sb[:, c, nsl],
                            start=(c == 0), stop=(c == 1),
                        )
                    if nch == 0:
                        nc.scalar.copy(o_sb[:, nsl], c_ps)
                    else:
                        nc.vector.tensor_copy(o_sb[:, nsl], c_ps)
                nc.sync.dma_start(out=out[msl, :], in_=o_sb)
            psC_ctx.__exit__(None, None, None)

    nc.compile()
    return nc


def _host_inputs(x, wq, wk, wv, wo, qn_w, kn_w):
    """Build the 8 per-core input maps."""
    # masks
    i = np.arange(128)[:, None]
    j = np.arange(KW)[None, :]
    m0 = np.where((j <= i) & (j < 128), 0.0, NEG).astype(np.float32)
    m1 = np.where((j <= i + 128) & (j < 256), 0.0, NEG).astype(np.float32)
    m2 = np.where((j >= i + 1) & (j <= i + 256), 0.0, NEG).astype(np.float32)
    masks = np.stack([m0, m1, m2])

    qn2 = (np.tile(qn_w, 2) * 0.125).astype(np.float32)[:, None]
    kn2 = np.tile(kn_w, 2).astype(np.float32)[:, None]

    xT = np.ascontiguousarray(np.transpose(x, (0, 2, 1)))  # [B, E, T]
    in_maps = []
    for core in range(NCORES):
        b, g = divmod(core, HG)
        cs = slice(g * COLS, (g + 1) * COLS)
        in_maps.append(
            {
                "xT": xT[b],
                "wq": np.ascontiguousarray(wq[:, cs]),
                "wk": np.ascontiguousarray(wk[:, cs]),
                "wv": np.ascontiguousarray(wv[:, cs]),
                "wo": np.ascontiguousarray(wo[cs, :]),
                "masks": masks,
                "qn2": qn2,
                "kn2": kn2,
            }
        )
    return in_maps


def run(trace=False, **inputs):
    if "nc" not in _cache:
        _cache["nc"] = _build()
    nc = _cache["nc"]
    in_maps = _host_inputs(
        np.asarray(inputs["x"]), np.asarray(inputs["wq"]), np.asarray(inputs["wk"]),
        np.asarray(inputs["wv"]), np.asarray(inputs["wo"]),
        np.asarray(inputs["qn_w"]), np.asarray(inputs["kn_w"]),
    )
    res = bass_utils.run_bass_kernel_spmd(
        nc, in_maps, core_ids=list(range(NCORES)), trace=trace
    )
    bo = np.asarray(inputs["bo"], np.float32)
    outs = []
    for b in range(B):
        acc = np.zeros((T, E), np.float32)
        for g in range(HG):
            acc += res.results[b * HG + g]["out"]
        outs.append(acc + bo[None, :])
    return np.stack(outs), res


def kernel(**inputs):
    out, _ = run(trace=False, **inputs)
    return out


# revision 13
# speedup vs baseline: 1.0747x; 1.0747x over previous
"""Trainium2 Bass kernel for nn_AblationAttention (sliding-window causal
attention, W=256, with per-head RMSNorm on q/k).

Key math fact: the reference's "genetic fitness" block adds log(fitness)[b,h,q]
to scores — a constant along the softmax (k) axis — so softmax is invariant to
it and the block is a no-op for the output. We compute plain sliding-window
causal attention.

Sharding: 8 cores = batch (2) x head-group (4 groups of 4 heads).
Each core: full x for its batch (pre-transposed on host), column-sliced
wq/wk/wv, row-sliced wo. Host sums the 4 head-group partials per batch.

v2: bf16 matmul operands (PSUM accumulation stays f32), merged q|k projection
(single N=512 matmul stream), copies balanced across ACT/DVE.
"""

import sys

sys.path.insert(0, "/opt/trn_rl_repo")

import numpy as np
import ml_dtypes

import concourse.bass as bass
import concourse.tile as tile
from concourse import bacc, mybir
from concourse import bass_utils
from concourse.masks import make_identity

# Problem constants (hardcoded per harness contract)
B, T, E, H, W = 2, 2048, 1024, 16, 256
D = E // H  # 64
NCORES = 8
HG = 4  # head-groups
HPG = H // HG  # heads per core = 4
COLS = HPG * D  # 256
EPS = float(np.finfo(np.float32).eps)
NEG = -30000.0  # additive mask value; exp(NEG) == 0.0
FP = mybir.dt.float32
BF = mybir.dt.bfloat16
NT = T // 128  # 16 query tiles
KW = 384  # key window width in columns (3 tiles)
AF = mybir.ActivationFunctionType

_cache = {}


def _build():
    nc = bacc.Bacc(
        "TRN2",
        target_bir_lowering=False,
        debug=False,
        enable_asserts=False,
        num_devices=NCORES,
    )
    xT = nc.dram_tensor("xT", [E, T], BF, kind="ExternalInput").ap()
    wqk = nc.dram_tensor("wqk", [E, 2 * COLS], BF, kind="ExternalInput").ap()
    wv = nc.dram_tensor("wv", [E, COLS], BF, kind="ExternalInput").ap()
    wo = nc.dram_tensor("wo", [COLS, E], BF, kind="ExternalInput").ap()
    masks = nc.dram_tensor("masks", [3, 128, 2, KW], FP, kind="ExternalInput").ap()
    qn2 = nc.dram_tensor("qn2", [128, 1], FP, kind="ExternalInput").ap()
    kn2 = nc.dram_tensor("kn2", [128, 1], FP, kind="ExternalInput").ap()
    out = nc.dram_tensor("out", [T, E], FP, kind="ExternalOutput").ap()

    with tile.TileContext(nc) as tc:
        with (
            tc.tile_pool(name="singles", bufs=1) as singles,
            tc.tile_pool(name="xin", bufs=4) as xin,
            tc.tile_pool(name="work", bufs=3) as work,
            tc.tile_pool(name="stats", bufs=4) as stats,
            tc.tile_pool(name="outst", bufs=3) as outst,
        ):
            # ---- resident tensors ----
            wqk_sb = singles.tile([128, 8, 2 * COLS], BF, tag="wqk")
            wv_sb = singles.tile([128, 8, COLS], BF, tag="wv")
            wo_sb = singles.tile([128, 2, E], BF, tag="wo")
            mask_sb = singles.tile([128, 3, 2, KW], FP, tag="masks")
            qn_sb = singles.tile([128, 1], FP, tag="qn2")
            kn_sb = singles.tile([128, 1], FP, tag="kn2")
            id_sb = singles.tile([128, 128], FP, tag="ident")
            idb_sb = singles.tile([128, 128], BF, tag="identb")
            eps_sb = singles.tile([128, 1], FP, tag="eps")
            qT_sb = singles.tile([128, 2, T], BF, tag="qT")
            kT_sb = singles.tile([128, 2, T], BF, tag="kT")
            v_sb = singles.tile([128, NT, COLS], BF, tag="vsb")
            hoT_sb = singles.tile([128, 2, T], BF, tag="hoT")

            nc.sync.dma_start(out=wqk_sb, in_=wqk.rearrange("(k p) c -> p k c", p=128))
            nc.sync.dma_start(out=wv_sb, in_=wv.rearrange("(k p) c -> p k c", p=128))
            nc.sync.dma_start(out=wo_sb, in_=wo.rearrange("(k p) e -> p k e", p=128))
            nc.sync.dma_start(out=mask_sb, in_=masks.rearrange("m p a j -> p m a j"))
            nc.sync.dma_start(out=qn_sb, in_=qn2)
            nc.sync.dma_start(out=kn_sb, in_=kn2)
            make_identity(nc, id_sb)
            make_identity(nc, idb_sb)
            nc.vector.memset(eps_sb, EPS)

            xT_r = xT.rearrange("(k p) t -> p k t", p=128)

            # ============ Phase A: qkv projections + rmsnorm + transpose ====
            psA_ctx = tc.tile_pool(name="psA", bufs=2, space="PSUM")
            psA = psA_ctx.__enter__()
            for tc_i in range(4):  # chunks of 512 t
                x_t = xin.tile([128, 8, 512], BF, tag="xT")
                nc.sync.dma_start(out=x_t, in_=xT_r[:, :, tc_i * 512 : (tc_i + 1) * 512])
                for ml in range(4):
                    m = tc_i * 4 + ml
                    sl = slice(ml * 128, (ml + 1) * 128)
                    qk_ps = psA.tile([128, 2 * COLS], FP, tag="qk_ps")
                    v_ps = psA.tile([128, COLS], FP, tag="v_ps")
                    for kc in range(8):
                        st, sp = (kc == 0), (kc == 7)
                        nc.tensor.matmul(qk_ps, x_t[:, kc, sl], wqk_sb[:, kc, :], start=st, stop=sp)
                        nc.tensor.matmul(v_ps, x_t[:, kc, sl], wv_sb[:, kc, :], start=st, stop=sp)
                    # v: straight copy to sbuf (bf16 cast)
                    nc.scalar.copy(v_sb[:, m, :], v_ps)
                    # rmsnorm stats for q and k together: one sq pass, one
                    # 8-group reduce, one sqrt, one reciprocal
                    sq = work.tile([128, 2 * COLS], FP, tag="sq")
                    nc.scalar.activation(sq, qk_ps, AF.Square)
                    ssq8 = stats.tile([128, 2 * HPG], FP, tag="ssq8")
                    nc.vector.tensor_reduce(
                        ssq8,
                        sq.rearrange("p (g d) -> p g d", g=2 * HPG),
                        axis=mybir.AxisListType.X,
                        op=mybir.AluOpType.add,
                    )
                    sd8 = stats.tile([128, 2 * HPG], FP, tag="sd8")
                    nc.scalar.activation(sd8, ssq8, AF.Sqrt, bias=eps_sb, scale=1.0 / D)
                    rstd8 = stats.tile([128, 2 * HPG], FP, tag="rstd8")
                    nc.vector.reciprocal(rstd8, sd8)
                    for qki, (w_ap, dst) in enumerate(((qn_sb, qT_sb), (kn_sb, kT_sb))):
                        t_ps = qk_ps[:, qki * COLS : (qki + 1) * COLS]
                        nrm = work.tile([128, COLS], BF, tag="nrm")
                        for g in range(HPG):
                            gs = slice(g * D, (g + 1) * D)
                            nc.vector.tensor_scalar_mul(
                                nrm[:, gs], t_ps[:, gs], rstd8[:, qki * HPG + g : qki * HPG + g + 1]
                            )
                        # transpose the two 128-col head-pair blocks into one
                        # psum tile, then one strided copy into [128, 2, T] dest
                        t_ps2 = psA.tile([128, 256], BF, tag="t_ps")
                        for hp in range(2):
                            nc.tensor.transpose(
                                t_ps2[:, hp * 128 : (hp + 1) * 128],
                                nrm[:, hp * 128 : (hp + 1) * 128], idb_sb,
                            )
                        dview = dst[:, :, m * 128 : (m + 1) * 128]
                        t2v = t_ps2.rearrange("p (a b) -> p a b", a=2)
                        if qki == 0:
                            nc.vector.tensor_scalar_mul(dview, t2v, w_ap)
                        else:
                            nc.scalar.activation(dview, t2v, AF.Copy, scale=w_ap)
            psA_ctx.__exit__(None, None, None)

            # ============ Phase B: banded attention ========================
            psB_ctx = tc.tile_pool(name="psB", bufs=2, space="PSUM")
            psB = psB_ctx.__enter__()
            for hp in range(2):
                for qt in range(NT):
                    win = 0 if qt < 2 else (qt - 2) * 128
                    mi = min(qt, 2)
                    qsl = slice(qt * 128, (qt + 1) * 128)
                    o_ps = psB.tile([128, 128], FP, tag="o_ps")
                    # both heads' scores in one 2-bank psum tile:
                    # h0 at cols [0:384], h1 at [512:896] (each within a bank)
                    s2_ps = psB.tile([128, 1024], FP, tag="s2_ps")
                    for hi in range(2):
                        po = slice(hi * 64, hi * 64 + 64)
                        nc.tensor.matmul(
                            s2_ps[:, hi * 512 : hi * 512 + KW],
                            qT_sb[po, hp, qsl], kT_sb[po, hp, win : win + KW],
                            start=True, stop=True,
                        )
                    sm2 = work.tile([128, 2, KW], BF, tag="sm2")
                    s2v = s2_ps.rearrange("p (a j) -> p a j", a=2)[:, :, 0:KW]
                    nc.vector.tensor_add(sm2, s2v, mask_sb[:, mi, :, :])
                    for hi in range(2):
                        h = hp * 2 + hi
                        po = slice(hi * 64, hi * 64 + 64)
                        p = work.tile([128, KW], BF, tag="p")
                        sums = stats.tile([128, 1], FP, tag="sums")
                        nc.scalar.activation(p, sm2[:, hi, :], AF.Exp, accum_out=sums)
                        rec = stats.tile([128, 1], FP, tag="rec")
                        nc.vector.reciprocal(rec, sums)
                        pn = work.tile([128, KW], BF, tag="pn")
                        nc.vector.tensor_scalar_mul(pn, p, rec)
                        pT3_ps = psB.tile([128, KW], BF, tag="pT3_ps")
                        for j in range(3):
                            nc.tensor.transpose(
                                pT3_ps[:, j * 128 : (j + 1) * 128],
                                pn[:, j * 128 : (j + 1) * 128], idb_sb,
                            )
                        pT3 = work.tile([128, KW], BF, tag="pT3")
                        if hi == 0:
                            nc.vector.tensor_copy(pT3, pT3_ps)
                        else:
                            nc.scalar.copy(pT3, pT3_ps)
                        for j in range(3):
                            kt = win // 128 + j
                            nc.tensor.matmul(
                                o_ps[po, :], v_sb[:, kt, h * 64 : (h + 1) * 64],
                                pT3[:, j * 128 : (j + 1) * 128],
                                start=(j == 0), stop=(j == 2),
                            )
                    nc.vector.tensor_copy(hoT_sb[:, hp, qsl], o_ps)
            psB_ctx.__exit__(None, None, None)

            # ============ Phase C: output projection =======================
            psC_ctx = tc.tile_pool(name="psC", bufs=2, space="PSUM")
            psC = psC_ctx.__enter__()
            for m in range(NT):
                msl = slice(m * 128, (m + 1) * 128)
                o_sb = outst.tile([128, E], FP, tag="o_sb")
                for nch in range(2):
                    nsl = slice(nch * 512, (nch + 1) * 512)
                    c_ps = psC.tile([128, 512], FP, tag="c_ps")
                    for c in range(2):
                        nc.tensor.matmul(
                            c_ps, hoT_sb[:, c, msl], wo_sb[:, c, nsl],
                            start=(c == 0), stop=(c == 1),
                        )
                    if nch == 0:
                        nc.scalar.copy(o_sb[:, nsl], c_ps)
                    else:
                        nc.vector.tensor_copy(o_sb[:, nsl], c_ps)
                nc.sync.dma_start(out=out[msl, :], in_=o_sb)
            psC_ctx.__exit__(None, None, None)

    nc.compile()
    return nc


def _host_inputs(x, wq, wk, wv, wo, qn_w, kn_w):
    """Build the 8 per-core input maps."""
    i = np.arange(128)[:, None]
    j = np.arange(KW)[None, :]
    m0 = np.where((j <= i) & (j < 128), 0.0, NEG).astype(np.float32)
    m1 = np.where((j <= i + 128) & (j < 256), 0.0, NEG).astype(np.float32)
    m2 = np.where((j >= i + 1) & (j <= i + 256), 0.0, NEG).astype(np.float32)
    masks = np.stack([m0, m1, m2])[:, :, None, :].repeat(2, axis=2)

    qn2 = (np.tile(qn_w, 2) * 0.125).astype(np.float32)[:, None]
    kn2 = np.tile(kn_w, 2).astype(np.float32)[:, None]

    bf = ml_dtypes.bfloat16
    xT = np.ascontiguousarray(np.transpose(x, (0, 2, 1))).astype(bf)  # [B, E, T]
    in_maps = []
    for core in range(NCORES):
        b, g = divmod(core, HG)
        cs = slice(g * COLS, (g + 1) * COLS)
        wqk = np.concatenate([wq[:, cs], wk[:, cs]], axis=1).astype(bf)
        in_maps.append(
            {
                "xT": xT[b],
                "wqk": np.ascontiguousarray(wqk),
                "wv": np.ascontiguousarray(wv[:, cs]).astype(bf),
                "wo": np.ascontiguousarray(wo[cs, :]).astype(bf),
                "masks": np.ascontiguousarray(masks),
                "qn2": qn2,
                "kn2": kn2,
            }
        )
    return in_maps


def run(trace=False, **inputs):
    if "nc" not in _cache:
        _cache["nc"] = _build()
    nc = _cache["nc"]
    in_maps = _host_inputs(
        np.asarray(inputs["x"]), np.asarray(inputs["wq"]), np.asarray(inputs["wk"]),
        np.asarray(inputs["wv"]), np.asarray(inputs["wo"]),
        np.asarray(inputs["qn_w"]), np.asarray(inputs["kn_w"]),
    )
    res = bass_utils.run_bass_kernel_spmd(
        nc, in_maps, core_ids=list(range(NCORES)), trace=trace
    )
    bo = np.asarray(inputs["bo"], np.float32)
    outs = []
    for b in range(B):
        acc = np.zeros((T, E), np.float32)
        for g in range(HG):
            acc += res.results[b * HG + g]["out"]
        outs.append(acc + bo[None, :])
    return np.stack(outs), res


def kernel(**inputs):
    out, _ = run(trace=False, **inputs)
    return out


# revision 33
# speedup vs baseline: 101.6848x; 94.6126x over previous
"""Trainium2 Bass kernel for nn_AblationAttention (sliding-window causal
attention, W=256, with per-head RMSNorm on q/k).

Key math fact: the reference's "genetic fitness" block adds log(fitness)[b,h,q]
to scores — a constant along the softmax (k) axis — so softmax is invariant to
it and the block is a no-op for the output. We compute plain sliding-window
causal attention.

Sharding: 8 cores = batch (2) x head-group (4 groups of 4 heads).
Each core: full x for its batch (pre-transposed on host), column-sliced
wq/wk/wv, row-sliced wo. Host sums the 4 head-group partials per batch.

v2: bf16 matmul operands (PSUM accumulation stays f32), merged q|k projection
(single N=512 matmul stream), copies balanced across ACT/DVE.
"""

import sys

sys.path.insert(0, "/opt/trn_rl_repo")

import numpy as np
import ml_dtypes

import concourse.bass as bass
import concourse.tile as tile
from concourse import bacc, mybir
from concourse import bass_utils
from concourse.masks import make_identity

# Problem constants (hardcoded per harness contract)
B, T, E, H, W = 2, 2048, 1024, 16, 256
D = E // H  # 64
NCORES = 8
HG = 4  # head-groups
HPG = H // HG  # heads per core = 4
COLS = HPG * D  # 256
EPS = float(np.finfo(np.float32).eps)
NEG = -30000.0  # additive mask value; exp(NEG) == 0.0
FP = mybir.dt.float32
BF = mybir.dt.bfloat16
NT = T // 128  # 16 query tiles
KW = 384  # key window width in columns (3 tiles)
AF = mybir.ActivationFunctionType

_cache = {}


def _build():
    nc = bacc.Bacc(
        "TRN2",
        target_bir_lowering=False,
        debug=False,
        enable_asserts=False,
        num_devices=NCORES,
    )
    xT = nc.dram_tensor("xT", [E, T], BF, kind="ExternalInput").ap()
    wqk = nc.dram_tensor("wqk", [E, 2 * COLS], BF, kind="ExternalInput").ap()
    wv = nc.dram_tensor("wv", [E, COLS], BF, kind="ExternalInput").ap()
    wo = nc.dram_tensor("wo", [COLS, E], BF, kind="ExternalInput").ap()
    masks = nc.dram_tensor("masks", [3, 128, 2, KW], FP, kind="ExternalInput").ap()
    qn2 = nc.dram_tensor("qn2", [128, 1], FP, kind="ExternalInput").ap()
    kn2 = nc.dram_tensor("kn2", [128, 1], FP, kind="ExternalInput").ap()
    out = nc.dram_tensor("out", [T, E], FP, kind="ExternalOutput").ap()

    with tile.TileContext(nc) as tc:
        with (
            tc.tile_pool(name="singles", bufs=1) as singles,
            tc.tile_pool(name="xin", bufs=4) as xin,
            tc.tile_pool(name="work", bufs=6) as work,
            tc.tile_pool(name="stats", bufs=12) as stats,
            tc.tile_pool(name="outst", bufs=3) as outst,
        ):
            # ---- resident tensors ----
            wqk_sb = singles.tile([128, 8, 2 * COLS], BF, tag="wqk")
            wv_sb = singles.tile([128, 8, COLS], BF, tag="wv")
            wo_sb = singles.tile([128, 2, E], BF, tag="wo")
            mask_sb = singles.tile([128, 3, 2, KW], FP, tag="masks")
            qn_sb = singles.tile([128, 1], FP, tag="qn2")
            kn_sb = singles.tile([128, 1], FP, tag="kn2")
            id_sb = singles.tile([128, 128], FP, tag="ident")
            idb_sb = singles.tile([128, 128], BF, tag="identb")
            eps_sb = singles.tile([128, 1], FP, tag="eps")
            qT_sb = singles.tile([128, 2, T], BF, tag="qT")
            kT_sb = singles.tile([128, 2, T], BF, tag="kT")
            v_sb = singles.tile([128, NT, COLS], BF, tag="vsb")
            hoT_sb = [singles.tile([128, 2, 512], BF, tag=f"hoT{g}", name=f"hoT{g}") for g in range(4)]

            nc.sync.dma_start(out=wqk_sb, in_=wqk.rearrange("(k p) c -> p k c", p=128))
            nc.sync.dma_start(out=wv_sb, in_=wv.rearrange("(k p) c -> p k c", p=128))
            nc.sync.dma_start(out=wo_sb, in_=wo.rearrange("(k p) e -> p k e", p=128))
            nc.sync.dma_start(out=mask_sb, in_=masks.rearrange("m p a j -> p m a j"))
            nc.sync.dma_start(out=qn_sb, in_=qn2)
            nc.sync.dma_start(out=kn_sb, in_=kn2)
            make_identity(nc, id_sb)
            make_identity(nc, idb_sb)
            nc.vector.memset(eps_sb, EPS)

            xT_r = xT.rearrange("(k p) t -> p k t", p=128)

            # ============ Phase A: qkv projections + rmsnorm + transpose ====
            psA_ctx = tc.tile_pool(name="psA", bufs=2, space="PSUM")
            psA = psA_ctx.__enter__()
            for tc_i in range(4):  # chunks of 512 t
                x_t = xin.tile([128, 8, 512], BF, tag="xT")
                nc.sync.dma_start(out=x_t, in_=xT_r[:, :, tc_i * 512 : (tc_i + 1) * 512])
                for ml in range(4):
                    m = tc_i * 4 + ml
                    sl = slice(ml * 128, (ml + 1) * 128)
                    qk_ps = psA.tile([128, 2 * COLS], FP, tag="qk_ps", bufs=3)
                    v_ps = psA.tile([128, COLS], FP, tag="v_ps")
                    for kc in range(8):
                        st, sp = (kc == 0), (kc == 7)
                        nc.tensor.matmul(qk_ps, x_t[:, kc, sl], wqk_sb[:, kc, :], start=st, stop=sp)
                        nc.tensor.matmul(v_ps, x_t[:, kc, sl], wv_sb[:, kc, :], start=st, stop=sp)
                    # v: straight copy to sbuf (bf16 cast)
                    nc.scalar.copy(v_sb[:, m, :], v_ps)
                    # rmsnorm stats for q and k together: one sq pass, one
                    # 8-group reduce, one sqrt, one reciprocal
                    sq = work.tile([128, 2 * COLS], FP, tag="sq")
                    nc.scalar.activation(sq, qk_ps, AF.Square)
                    ssq8 = stats.tile([128, 2 * HPG], FP, tag="ssq8")
                    nc.vector.tensor_reduce(
                        ssq8,
                        sq.rearrange("p (g d) -> p g d", g=2 * HPG),
                        axis=mybir.AxisListType.X,
                        op=mybir.AluOpType.add,
                    )
                    sd8 = stats.tile([128, 2 * HPG], FP, tag="sd8")
                    nc.scalar.activation(sd8, ssq8, AF.Sqrt, bias=eps_sb, scale=1.0 / D)
                    rstd8 = stats.tile([128, 2 * HPG], FP, tag="rstd8")
                    nc.vector.reciprocal(rstd8, sd8)
                    for qki, (w_ap, dst) in enumerate(((qn_sb, qT_sb), (kn_sb, kT_sb))):
                        t_ps = qk_ps[:, qki * COLS : (qki + 1) * COLS]
                        nrm = work.tile([128, COLS], BF, tag="nrm")
                        rsl = rstd8[:, qki * HPG : (qki + 1) * HPG]
                        rstd_b = bass.AP(
                            tensor=rsl.tensor,
                            offset=rsl.offset,
                            ap=[rsl.ap[0], [rsl.ap[1][0], HPG], [0, D]],
                        )
                        nc.vector.tensor_mul(
                            nrm.rearrange("p (g d) -> p g d", g=HPG),
                            t_ps.rearrange("p (g d) -> p g d", g=HPG),
                            rstd_b,
                        )
                        # transpose the two 128-col head-pair blocks into one
                        # psum tile, then one strided copy into [128, 2, T] dest
                        t_ps2 = psA.tile([128, 256], BF, tag="t_ps", bufs=3)
                        for hp in range(2):
                            nc.tensor.transpose(
                                t_ps2[:, hp * 128 : (hp + 1) * 128],
                                nrm[:, hp * 128 : (hp + 1) * 128], idb_sb,
                            )
                        dview = dst[:, :, m * 128 : (m + 1) * 128]
                        t2v = t_ps2.rearrange("p (a b) -> p a b", a=2)
                        if qki == 0:
                            nc.vector.tensor_scalar_mul(dview, t2v, w_ap)
                        else:
                            nc.scalar.activation(dview, t2v, AF.Copy, scale=w_ap)
            psA_ctx.__exit__(None, None, None)

            # ============ Phase B: banded attention ========================
            # Software-pipelined: iteration i+1's scores matmul is emitted
            # before iteration i's elementwise tail so PE never stalls on the
            # mask/exp/normalize chain.
            psB_ctx = tc.tile_pool(name="psB", bufs=2, space="PSUM")
            psB = psB_ctx.__enter__()

            iters = [(qt, hp, hi) for qt in range(NT) for hp in range(2) for hi in range(2)]
            o_ps_map = {}

            def emit_scores(it):
                qt, hp, hi = it
                win = 0 if qt < 2 else (qt - 2) * 128
                qsl = slice(qt * 128, (qt + 1) * 128)
                po = slice(hi * 64, hi * 64 + 64)
                s_ps = psB.tile([128, KW], FP, tag="s_ps", bufs=3, name="s_ps")
                nc.tensor.matmul(
                    s_ps, qT_sb[po, hp, qsl], kT_sb[po, hp, win : win + KW],
                    start=True, stop=True,
                )
                return s_ps

            def emit_tail(it, s_ps):
                qt, hp, hi = it
                win = 0 if qt < 2 else (qt - 2) * 128
                mi = min(qt, 2)
                qsl = slice(qt * 128, (qt + 1) * 128)
                h = hp * 2 + hi
                po = slice(hi * 64, hi * 64 + 64)
                sm = work.tile([128, KW], BF, tag="sm", name="sm")
                nc.vector.tensor_add(sm, s_ps, mask_sb[:, mi, 0, :])
                p = work.tile([128, KW], BF, tag="p", name="p")
                sums = stats.tile([128, 1], FP, tag="sums", name="sums")
                nc.scalar.activation(p, sm, AF.Exp, accum_out=sums)
                rec = stats.tile([128, 1], FP, tag="rec", name="rec")
                nc.vector.reciprocal(rec, sums)
                pn = work.tile([128, KW], BF, tag="pn", name="pn")
                nc.vector.tensor_scalar_mul(pn, p, rec)
                pT3_ps = psB.tile([128, KW], BF, tag="pT3_ps", bufs=3, name="pT3_ps")
                for j in range(3):
                    nc.tensor.transpose(
                        pT3_ps[:, j * 128 : (j + 1) * 128],
                        pn[:, j * 128 : (j + 1) * 128], idb_sb,
                    )
                pT3 = work.tile([128, KW], BF, tag="pT3", name="pT3")
                if hi == 0:
                    nc.vector.tensor_copy(pT3, pT3_ps)
                else:
                    nc.scalar.copy(pT3, pT3_ps)
                if (qt, hp) not in o_ps_map:
                    o_ps_map[(qt, hp)] = psB.tile([128, 128], FP, tag="o_ps", bufs=2, name="o_ps")
                o_ps = o_ps_map[(qt, hp)]
                for j in range(3):
                    kt = win // 128 + j
                    nc.tensor.matmul(
                        o_ps[po, :], v_sb[:, kt, h * 64 : (h + 1) * 64],
                        pT3[:, j * 128 : (j + 1) * 128],
                        start=(j == 0), stop=(j == 2),
                    )
                if hi == 1:
                    nc.vector.tensor_copy(
                        hoT_sb[qt // 4][:, hp, (qt % 4) * 128 : (qt % 4 + 1) * 128], o_ps
                    )
                    del o_ps_map[(qt, hp)]

            pending = []
            import os
            LOOKAHEAD = int(os.environ.get("KLOOK", "2"))
            for it in iters:
                pending.append((it, emit_scores(it)))
                if len(pending) > LOOKAHEAD:
                    pit, ps = pending.pop(0)
                    emit_tail(pit, ps)
            for pit, ps in pending:
                emit_tail(pit, ps)

            psB_ctx.__exit__(None, None, None)

            # ============ Phase C: output projection =======================
            psC_ctx = tc.tile_pool(name="psC", bufs=2, space="PSUM")
            psC = psC_ctx.__enter__()
            for m in range(NT):
                g, r = m // 4, m % 4
                rsl = slice(r * 128, (r + 1) * 128)
                o_sb = outst.tile([128, E], FP, tag="o_sb")
                for nch in range(2):
                    nsl = slice(nch * 512, (nch + 1) * 512)
                    c_ps = psC.tile([128, 512], FP, tag="c_ps", bufs=2)
                    for c in range(2):
                        nc.tensor.matmul(
                            c_ps, hoT_sb[g][:, c, rsl], wo_sb[:, c, nsl],
                            start=(c == 0), stop=(c == 1),
                        )
                    if nch == 0:
                        nc.scalar.copy(o_sb[:, nsl], c_ps)
                    else:
                        nc.vector.tensor_copy(o_sb[:, nsl], c_ps)
                nc.sync.dma_start(out=out[m * 128 : (m + 1) * 128, :], in_=o_sb)
            psC_ctx.__exit__(None, None, None)

    nc.compile()
    return nc


def _host_inputs(x, wq, wk, wv, wo, qn_w, kn_w):
    """Build the 8 per-core input maps."""
    i = np.arange(128)[:, None]
    j = np.arange(KW)[None, :]
    m0 = np.where((j <= i) & (j < 128), 0.0, NEG).astype(np.float32)
    m1 = np.where((j <= i + 128) & (j < 256), 0.0, NEG).astype(np.float32)
    m2 = np.where((j >= i + 1) & (j <= i + 256), 0.0, NEG).astype(np.float32)
    masks = np.stack([m0, m1, m2])[:, :, None, :].repeat(2, axis=2)

    qn2 = (np.tile(qn_w, 2) * 0.125).astype(np.float32)[:, None]
    kn2 = np.tile(kn_w, 2).astype(np.float32)[:, None]

    bf = ml_dtypes.bfloat16
    xT = np.ascontiguousarray(np.transpose(x, (0, 2, 1))).astype(bf)  # [B, E, T]
    in_maps = []
    for core in range(NCORES):
        b, g = divmod(core, HG)
        cs = slice(g * COLS, (g + 1) * COLS)
        wqk = np.concatenate([wq[:, cs], wk[:, cs]], axis=1).astype(bf)
        in_maps.append(
            {
                "xT": xT[b],
                "wqk": np.ascontiguousarray(wqk),
                "wv": np.ascontiguousarray(wv[:, cs]).astype(bf),
                "wo": np.ascontiguousarray(wo[cs, :]).astype(bf),
                "masks": np.ascontiguousarray(masks),
                "qn2": qn2,
                "kn2": kn2,
            }
        )
    return in_maps


def run(trace=False, **inputs):
    if "nc" not in _cache:
        _cache["nc"] = _build()
    nc = _cache["nc"]
    in_maps = _host_inputs(
        np.asarray(inputs["x"]), np.asarray(inputs["wq"]), np.asarray(inputs["wk"]),
        np.asarray(inputs["wv"]), np.asarray(inputs["wo"]),
        np.asarray(inputs["qn_w"]), np.asarray(inputs["kn_w"]),
    )
    res = bass_utils.run_bass_kernel_spmd(
        nc, in_maps, core_ids=list(range(NCORES)), trace=trace
    )
    bo = np.asarray(inputs["bo"], np.float32)
    outs = []
    for b in range(B):
        acc = np.zeros((T, E), np.float32)
        for g in range(HG):
            acc += res.results[b * HG + g]["out"]
        outs.append(acc + bo[None, :])
    return np.stack(outs), res


def kernel(**inputs):
    out, _ = run(trace=False, **inputs)
    return out
